# revision 15
# baseline (speedup 1.0000x reference)
"""ChunkedAttention (nn_ChunkedAttention_43568148251092) Trainium2 kernel.

Full inputs q/k/v: [1, 4096, 16, 128] fp32. Shards the 16 heads across the
8 NeuronCores (2 heads per core, pure head parallelism — no collectives),
runs a Bass/Tile attention kernel per core, and concatenates the results.

Per-head pipeline on each core (S=4096 tokens, D=128):
  - int8 quant-dequant of K and V per token, trunc-toward-zero exactly as the
    reference. Trunc via sign-offset + RNE int convert: t = (x>0)*0.9998;
    i = rne_i32(x + 0.4999 - t). Exact for integer x (incl +/-127); off-by-one
    only for |frac| within 1e-4 of 1, which is numerically negligible.
    Kint kept as fp16 integers (exact: |int| <= 127); per-token kscale kept
    fp32 and folded into the softmax exp via the ACT per-partition scale.
    V dequantized to bf16 (Vint * vscale).
  - Q cast to fp16; Q and Kint transposed to [d, s] via PE transpose.
  - S^T[k,q] = Kint^T.T @ Q^T in PSUM fp32 (single 128-deep matmul).
  - P'[k,q] = exp(kscale/sqrt(D) * S^T - 40) via ScalarE (bias keeps the
    bf16 range safe without a row-max pass; scaled scores are ~N(0,1)).
  - out[q, 0:128|denom] = sum_kt P'_kt.T @ [Vdq | ones] accumulated in PSUM;
    the appended ones-column yields the softmax denominator for free.
  - out = out[:, :128] * (1/denom) per partition, DMA to DRAM.

Scheduling structure (the point of this rewrite vs the first version):
  - per-head big tiles are double-buffered so head h+1's preprocessing
    (DVE-heavy) overlaps head h's main loop (ACT-heavy exp is the
    bottleneck engine: 256 x [128,1024] exps ~= 266us busy).
  - PSUM->SBUF transpose copies go explicitly to DVE (nc.any would put
    them on ScalarE, polluting the bottleneck engine).
  - Q fp32->fp16 casts and Vext ones-memsets go to GpSimd (idle; SBUF-only).
  - PV matmuls for qc are interleaved into the QK/exp stream of qc+1 so
    PE stays dense while ACT runs exps.
  - output stores batched 512 tokens per DMA.
"""

import math
import time

import numpy as np

import concourse.bass as bass
import concourse.mybir as mybir
import concourse.tile as tile
from concourse import bacc
from concourse.bass_utils import run_bass_kernel_spmd
from concourse.masks import make_identity

F32 = mybir.dt.float32
BF16 = mybir.dt.bfloat16
FP16 = mybir.dt.float16
I32 = mybir.dt.int32
AX = mybir.AxisListType.X
OP = mybir.AluOpType
EXP = mybir.ActivationFunctionType.Exp

_S = 4096
_H_TOTAL = 16
_D = 128
_N_CORES = 8
_H = _H_TOTAL // _N_CORES  # heads per core

_NC_CACHE = {}


def _bcast3(ap2, n):
    """[128, J] AP -> [128, J, n] broadcast AP (inner stride 0)."""
    return bass.AP(tensor=ap2.tensor, offset=ap2.offset, ap=[*ap2.ap, [0, n]])


def _build_nc(S=_S, H=_H, D=_D, qk_dt=FP16, pp_bufs=64, ld_bufs=6,
              psS_bufs=2, psT_bufs=1, psO_bufs=3, pre_emit="qc0",
              pv_chunk=4, cast_eng="gpsimd", ones_eng="gpsimd"):
    assert D == 128 and S % 512 == 0
    n_kt = S // 128          # 32 key tiles of 128 tokens
    n_ch = S // 512          # 8 chunks of 512 tokens
    n_qc = S // 1024         # 4 query column blocks of 1024

    nc = bacc.Bacc("TRN2")
    q_d = nc.dram_tensor("q", [S, H, D], F32, kind="ExternalInput")
    k_d = nc.dram_tensor("k", [S, H, D], F32, kind="ExternalInput")
    v_d = nc.dram_tensor("v", [S, H, D], F32, kind="ExternalInput")
    o_d = nc.dram_tensor("o", [S, H, D], F32, kind="ExternalOutput")

    with tile.TileContext(nc) as tc:
        with (
            tc.tile_pool(name="const", bufs=1) as constp,
            tc.tile_pool(name="big", bufs=2) as bigp,
            tc.tile_pool(name="ld", bufs=ld_bufs) as ldp,
            tc.tile_pool(name="tmp", bufs=2) as tmpp,
            tc.tile_pool(name="b16", bufs=2) as b16p,
            tc.tile_pool(name="small", bufs=8) as smallp,
            tc.tile_pool(name="pp", bufs=pp_bufs) as ppool,
            tc.tile_pool(name="outp", bufs=2) as outp,
            tc.tile_pool(name="psT", bufs=psT_bufs, space="PSUM") as psT,
            tc.tile_pool(name="psS", bufs=psS_bufs, space="PSUM") as psS,
            tc.tile_pool(name="psO", bufs=psO_bufs, space="PSUM") as psO,
        ):
            bias_t = constp.tile([128, 1], F32)
            nc.vector.memset(bias_t[:], -40.0)
            # Dummy activation emitted first so the ACT table load happens
            # at t~0 instead of just before the first real exp.
            warm = constp.tile([128, 1], F32)
            nc.scalar.activation(warm[:], bias_t[:], EXP, bias=bias_t[:])
            ident32 = constp.tile([128, 128], F32)
            make_identity(nc, ident32[:])
            ident16 = constp.tile([128, 128], qk_dt)
            nc.vector.tensor_copy(ident16[:], ident32[:])
            ceng = getattr(nc, cast_eng)
            oeng = getattr(nc, ones_eng)

            def emit_k(h, hd, c):
                """Quantize K for chunk c (512 tokens) of head h."""
                s0 = c * 512
                kf = ldp.tile([128, 4, 128], F32, tag="ld")
                nc.sync.dma_start(
                    out=kf[:],
                    in_=k_d[s0:s0 + 512, h, :].rearrange(
                        "(j p) d -> p j d", p=128))
                am = smallp.tile([128, 4], F32, tag="am")
                nc.vector.reduce_max(am[:], kf[:], axis=AX,
                                     apply_absolute_value=True)
                sc = smallp.tile([128, 4], F32, tag="sc")
                nc.vector.tensor_scalar(sc[:], am[:], 1e-8, 1.0 / 127.0,
                                        op0=OP.max, op1=OP.mult)
                ks = hd["ks"][c]
                nc.vector.tensor_scalar(ks[:], sc[:], 1.0 / math.sqrt(128.0),
                                        None, op0=OP.mult)
                rc = smallp.tile([128, 4], F32, tag="rc")
                nc.vector.reciprocal(rc[:], sc[:])
                nc.vector.tensor_tensor(kf[:], kf[:], _bcast3(rc[:], 128),
                                        op=OP.mult)  # x, in-place
                t = tmpp.tile([128, 4, 128], F32, tag="t")
                nc.vector.tensor_scalar(t[:], kf[:], 0.0, 0.9998,
                                        op0=OP.is_gt, op1=OP.mult)
                i32 = tmpp.tile([128, 4, 128], I32, tag="i32")
                nc.vector.scalar_tensor_tensor(i32[:], kf[:], 0.4999, t[:],
                                               op0=OP.add, op1=OP.subtract)
                k16 = b16p.tile([128, 4, 128], qk_dt, tag="k16")
                nc.vector.tensor_copy(k16[:], i32[:])
                pst = psT.tile([128, 4, 128], qk_dt, tag="pst")
                for j in range(4):
                    nc.tensor.transpose(pst[:, j, :], k16[:, j, :], ident16[:])
                nc.vector.tensor_copy(hd["KT"][c][:], pst[:])

            def emit_q(h, hd, c):
                """Cast+transpose Q for chunk c (512 tokens) of head h."""
                s0 = c * 512
                qf = ldp.tile([128, 4, 128], F32, tag="ld")
                nc.sync.dma_start(
                    out=qf[:],
                    in_=q_d[s0:s0 + 512, h, :].rearrange(
                        "(j p) d -> p j d", p=128))
                q16 = b16p.tile([128, 4, 128], qk_dt, tag="q16")
                ceng.tensor_copy(q16[:], qf[:])
                pst2 = psT.tile([128, 4, 128], qk_dt, tag="pst")
                for j in range(4):
                    nc.tensor.transpose(pst2[:, j, :], q16[:, j, :],
                                        ident16[:])
                nc.vector.tensor_copy(hd["QT"][c][:], pst2[:])

            def emit_kq(h, hd, c):
                emit_k(h, hd, c)
                emit_q(h, hd, c)

            def emit_v(h, hd, c):
                """Quantize-dequantize V chunk c of head h into Vext."""
                s0 = c * 512
                vf = ldp.tile([128, 4, 128], F32, tag="ld")
                nc.sync.dma_start(
                    out=vf[:],
                    in_=v_d[s0:s0 + 512, h, :].rearrange(
                        "(j p) d -> p j d", p=128))
                am2 = smallp.tile([128, 4], F32, tag="am")
                nc.vector.reduce_max(am2[:], vf[:], axis=AX,
                                     apply_absolute_value=True)
                sc2 = smallp.tile([128, 4], F32, tag="sc")
                nc.vector.tensor_scalar(sc2[:], am2[:], 1e-8, 1.0 / 127.0,
                                        op0=OP.max, op1=OP.mult)
                rc2 = smallp.tile([128, 4], F32, tag="rc")
                nc.vector.reciprocal(rc2[:], sc2[:])
                nc.vector.tensor_tensor(vf[:], vf[:], _bcast3(rc2[:], 128),
                                        op=OP.mult)
                t2 = tmpp.tile([128, 4, 128], F32, tag="t")
                nc.vector.tensor_scalar(t2[:], vf[:], 0.0, 0.9998,
                                        op0=OP.is_gt, op1=OP.mult)
                i32v = tmpp.tile([128, 4, 128], I32, tag="i32")
                nc.vector.scalar_tensor_tensor(i32v[:], vf[:], 0.4999, t2[:],
                                               op0=OP.add, op1=OP.subtract)
                vext = hd["V"]
                nc.vector.tensor_tensor(
                    vext[:, 4 * c:4 * c + 4, 0:128], i32v[:],
                    _bcast3(sc2[:], 128), op=OP.mult)
                oeng.memset(vext[:, 4 * c:4 * c + 4, 128:129], 1.0)

            def make_hd(h):
                return {
                    "KT": [bigp.tile([128, 512], qk_dt, tag=f"KT{c}",
                                     name=f"KT{c}") for c in range(n_ch)],
                    "QT": [bigp.tile([128, 512], qk_dt, tag=f"QT{c}",
                                     name=f"QT{c}") for c in range(n_ch)],
                    "V": bigp.tile([128, n_kt, 129], BF16, tag="V", name="V"),
                    "ks": [bigp.tile([128, 4], F32, tag=f"ks{c}",
                                     name=f"ks{c}") for c in range(n_ch)],
                    "h": h,
                }

            def emit_pv(prev, j):
                """Attention-weighted V for query tile j (128 q) of a
                completed (head, qc) score block, + denom normalize."""
                pts, vext, h, qc = prev
                ops_ = psO.tile([128, 129], F32, tag="ops")
                for kt in range(n_kt):
                    nc.tensor.matmul(
                        ops_[:], pts[kt][:, j * 128:(j + 1) * 128],
                        vext[:, kt, 0:129],
                        start=(kt == 0), stop=(kt == n_kt - 1))
                rcp = smallp.tile([128, 1], F32, tag="rcp")
                nc.vector.reciprocal(rcp[:], ops_[:, 128:129])
                if j % 4 == 0:
                    prev_ot[0] = outp.tile([128, 4, 128], F32, tag="ot", name="ot")
                ot = prev_ot[0]
                nc.vector.tensor_scalar(ot[:, j % 4, :], ops_[:, 0:128],
                                        rcp[:], None, op0=OP.mult)
                if j % 4 == 3:
                    q0 = qc * 1024 + (j - 3) * 128
                    nc.sync.dma_start(
                        out=o_d[q0:q0 + 512, h, :].rearrange(
                            "(j p) d -> p j d", p=128),
                        in_=ot[:])

            # Emission schedule: hooks[(h, qc, kt)] -> list of pre-work
            # thunks, spreading head h+1's preprocessing across head h's
            # main loop so the static per-engine schedule zippers instead
            # of head-of-line blocking.  K+Q chunks of head h are needed
            # from that head's qc0; V only from its qc1 (first PV).
            hds = {0: make_hd(0)}
            hooks = {}
            kq_pos = [(1, 4), (1, 14), (1, 24), (2, 2), (2, 12), (2, 22),
                      (3, 0), (3, 8)]
            v_pos = [(3, 16), (3, 24)] + [(4, kt) for kt in
                                          (2, 7, 12, 17, 22, 27)]
            for h in range(1, H):
                hds[h] = make_hd(h)
                for c in range(n_ch):
                    qc, kt = kq_pos[c]
                    hooks.setdefault((h - 1, qc, kt), []).append(
                        (emit_kq, h, c))
                for c in range(n_ch):
                    qc, kt = v_pos[c]
                    hq, hqc = (h - 1, qc) if qc < n_qc else (h, qc - n_qc)
                    hooks.setdefault((hq, hqc, kt), []).append(
                        (emit_v, h, c))

            # Head-0 prologue: Q runs one chunk ahead of K (the first QK
            # matmul needs QT chunks 0 AND 1, but only KT chunk 0), V last
            # (first needed by PV at qc1).
            emit_k(0, hds[0], 0)
            emit_q(0, hds[0], 0)
            emit_q(0, hds[0], 1)
            for c in range(1, n_ch):
                emit_k(0, hds[0], c)
                if c + 1 < n_ch:
                    emit_q(0, hds[0], c + 1)
            for c in range(n_ch):
                emit_v(0, hds[0], c)

            prev = None          # completed (pts, vext, h, qc) awaiting PV
            prev_ot = [None]
            for h in range(H):
                hd = hds[h]
                for qc in range(n_qc):
                    pts = []
                    for kt in range(n_kt):
                        for fn, hh, cc in hooks.get((h, qc, kt), ()):
                            fn(hh, hds[hh], cc)
                        if (prev is not None and kt % pv_chunk == 0
                                and kt // pv_chunk < 8):
                            emit_pv(prev, kt // pv_chunk)
                        sps = psS.tile([128, 1024], F32, tag="sps")
                        w = hd["KT"][kt // 4][:, (kt % 4) * 128:
                                              (kt % 4 + 1) * 128]
                        for half in range(2):
                            nc.tensor.matmul(
                                sps[:, half * 512:(half + 1) * 512], w,
                                hd["QT"][2 * qc + half][:],
                                start=True, stop=True)
                        pt = ppool.tile([128, 1024], BF16, tag="pp")
                        ksl = hd["ks"][kt // 4]
                        nc.scalar.activation(pt[:], sps[:], EXP,
                                             bias=bias_t[:],
                                             scale=ksl[:, kt % 4:kt % 4 + 1])
                        pts.append(pt)
                    prev = (pts, hd["V"], hd["h"], qc)
            for j in range(8):
                emit_pv(prev, j)

    nc.compile()
    return nc


def get_nc(**kwargs):
    key = tuple(sorted(kwargs.items()))
    if key not in _NC_CACHE:
        _NC_CACHE[key] = _build_nc(**kwargs)
    return _NC_CACHE[key]


def kernel(q, k, v, _trace=False, _trace_cores=None, _nc_kwargs=None):
    """Full-input entry point: q/k/v [1, 4096, 16, 128] fp32 -> same shape."""
    assert q.shape == (1, _S, _H_TOTAL, _D), q.shape
    nc = get_nc(**(_nc_kwargs or {}))
    in_maps = []
    for c in range(_N_CORES):
        hs = slice(c * _H, (c + 1) * _H)
        in_maps.append({
            "q": np.ascontiguousarray(q[0, :, hs, :], dtype=np.float32),
            "k": np.ascontiguousarray(k[0, :, hs, :], dtype=np.float32),
            "v": np.ascontiguousarray(v[0, :, hs, :], dtype=np.float32),
        })
    # The axon-tunneled device occasionally reports a transient
    # NRT_EXEC_UNIT_UNRECOVERABLE on the first execution; a retry succeeds.
    last_err = None
    for attempt in range(3):
        try:
            res = run_bass_kernel_spmd(nc, in_maps,
                                       core_ids=list(range(_N_CORES)),
                                       trace=_trace, trace_cores=_trace_cores)
            break
        except Exception as e:  # noqa: BLE001
            last_err = e
            time.sleep(2.0 * (attempt + 1))
    else:
        raise last_err
    out = np.concatenate([res.results[c]["o"] for c in range(_N_CORES)],
                         axis=1)[None]
    out = np.ascontiguousarray(out, dtype=np.float32)
    if _trace:
        return out, res
    return out


# revision 28
# speedup vs baseline: 1.0013x; 1.0013x over previous
"""ChunkedAttention (nn_ChunkedAttention_43568148251092) Trainium2 kernel.

Full inputs q/k/v: [1, 4096, 16, 128] fp32. Shards the 16 heads across the
8 NeuronCores (2 heads per core, pure head parallelism — no collectives),
runs a Bass/Tile attention kernel per core, and concatenates the results.

Per-head pipeline on each core (S=4096 tokens, D=128):
  - int8 quant-dequant of K and V per token, trunc-toward-zero exactly as the
    reference. Trunc via sign-offset + RNE int convert: t = (x>0)*0.9998;
    i = rne_i32(x + 0.4999 - t). Exact for integer x (incl +/-127); off-by-one
    only for |frac| within 1e-4 of 1, which is numerically negligible.
    Kint kept as fp16 integers (exact: |int| <= 127); per-token kscale kept
    fp32 and folded into the softmax exp via the ACT per-partition scale.
    V dequantized to bf16 (Vint * vscale).
  - Q cast to fp16; Q and Kint transposed to [d, s] via PE transpose.
  - S^T[k,q] = Kint^T.T @ Q^T in PSUM fp32 (single 128-deep matmul).
  - P'[k,q] = exp(kscale/sqrt(D) * S^T - 40) via ScalarE (bias keeps the
    bf16 range safe without a row-max pass; scaled scores are ~N(0,1)).
  - out[q, 0:128|denom] = sum_kt P'_kt.T @ [Vdq | ones] accumulated in PSUM;
    the appended ones-column yields the softmax denominator for free.
  - out = out[:, :128] * (1/denom) per partition, DMA to DRAM.

Scheduling structure (the point of this rewrite vs the first version):
  - per-head big tiles are double-buffered so head h+1's preprocessing
    (DVE-heavy) overlaps head h's main loop (ACT-heavy exp is the
    bottleneck engine: 256 x [128,1024] exps ~= 266us busy).
  - PSUM->SBUF transpose copies go explicitly to DVE (nc.any would put
    them on ScalarE, polluting the bottleneck engine).
  - Q fp32->fp16 casts and Vext ones-memsets go to GpSimd (idle; SBUF-only).
  - PV matmuls for qc are interleaved into the QK/exp stream of qc+1 so
    PE stays dense while ACT runs exps.
  - output stores batched 512 tokens per DMA.
"""

import math
import time

import numpy as np

import concourse.bass as bass
import concourse.mybir as mybir
import concourse.tile as tile
from concourse import bacc
from concourse.bass_utils import run_bass_kernel_spmd
from concourse.masks import make_identity

F32 = mybir.dt.float32
BF16 = mybir.dt.bfloat16
FP16 = mybir.dt.float16
I32 = mybir.dt.int32
AX = mybir.AxisListType.X
OP = mybir.AluOpType
EXP = mybir.ActivationFunctionType.Exp

_S = 4096
_H_TOTAL = 16
_D = 128
_N_CORES = 8
_H = _H_TOTAL // _N_CORES  # heads per core

_NC_CACHE = {}


def _bcast3(ap2, n):
    """[128, J] AP -> [128, J, n] broadcast AP (inner stride 0)."""
    return bass.AP(tensor=ap2.tensor, offset=ap2.offset, ap=[*ap2.ap, [0, n]])


def _build_nc(S=_S, H=_H, D=_D, qk_dt=FP16, pp_bufs=63, ld_bufs=6,
              psS_bufs=2, psT_bufs=1, psO_bufs=3, pre_emit="qc0",
              pv_chunk=4, cast_eng="gpsimd", ones_eng="gpsimd",
              split_first=True, v_pool=0):
    assert D == 128 and S % 512 == 0
    n_kt = S // 128          # 32 key tiles of 128 tokens
    n_ch = S // 512          # 8 chunks of 512 tokens
    n_qc = S // 1024         # 4 query column blocks of 1024

    nc = bacc.Bacc("TRN2")
    q_d = nc.dram_tensor("q", [S, H, D], F32, kind="ExternalInput")
    k_d = nc.dram_tensor("k", [S, H, D], F32, kind="ExternalInput")
    v_d = nc.dram_tensor("v", [S, H, D], F32, kind="ExternalInput")
    o_d = nc.dram_tensor("o", [S, H, D], F32, kind="ExternalOutput")

    with tile.TileContext(nc) as tc:
        with (
            tc.tile_pool(name="const", bufs=1) as constp,
            tc.tile_pool(name="big", bufs=2) as bigp,
            tc.tile_pool(name="ld", bufs=ld_bufs) as ldp,
            tc.tile_pool(name="tmp", bufs=2) as tmpp,
            tc.tile_pool(name="b16", bufs=2) as b16p,
            tc.tile_pool(name="small", bufs=8) as smallp,
            tc.tile_pool(name="pp", bufs=pp_bufs) as ppool,
            tc.tile_pool(name="outp", bufs=2) as outp,
            tc.tile_pool(name="psT", bufs=psT_bufs, space="PSUM") as psT,
            tc.tile_pool(name="psS", bufs=psS_bufs, space="PSUM") as psS,
            tc.tile_pool(name="psO", bufs=psO_bufs, space="PSUM") as psO,
        ):
            bias_t = constp.tile([128, 1], F32)
            nc.vector.memset(bias_t[:], -40.0)
            # Dummy activation emitted first so the ACT table load happens
            # at t~0 instead of just before the first real exp.
            warm = constp.tile([128, 1], F32)
            nc.scalar.activation(warm[:], bias_t[:], EXP, bias=bias_t[:])
            ident32 = constp.tile([128, 128], F32)
            make_identity(nc, ident32[:])
            ident16 = constp.tile([128, 128], qk_dt)
            nc.vector.tensor_copy(ident16[:], ident32[:])
            ceng = getattr(nc, cast_eng)
            oeng = getattr(nc, ones_eng)

            def emit_k(h, hd, c, j0=0, nj=4, dest=None, ksdest=None):
                """Quantize K tokens [c*512+j0*128, +nj*128) of head h into
                dest (default: the chunk's KT tile) + kscale tile."""
                s0 = c * 512 + j0 * 128
                kf = ldp.tile([128, nj, 128], F32, tag="ld", name="kf")
                nc.sync.dma_start(
                    out=kf[:],
                    in_=k_d[s0:s0 + nj * 128, h, :].rearrange(
                        "(j p) d -> p j d", p=128))
                am = smallp.tile([128, nj], F32, tag="am", name="am")
                nc.vector.reduce_max(am[:], kf[:], axis=AX,
                                     apply_absolute_value=True)
                sc = smallp.tile([128, nj], F32, tag="sc", name="sc")
                nc.vector.tensor_scalar(sc[:], am[:], 1e-8, 1.0 / 127.0,
                                        op0=OP.max, op1=OP.mult)
                ks = ksdest if ksdest is not None else hd["ks"][c]
                nc.vector.tensor_scalar(ks[:], sc[:], 1.0 / math.sqrt(128.0),
                                        None, op0=OP.mult)
                rc = smallp.tile([128, nj], F32, tag="rc", name="rc")
                nc.vector.reciprocal(rc[:], sc[:])
                nc.vector.tensor_tensor(kf[:], kf[:], _bcast3(rc[:], 128),
                                        op=OP.mult)  # x, in-place
                t = tmpp.tile([128, nj, 128], F32, tag="t", name="t")
                nc.vector.tensor_scalar(t[:], kf[:], 0.0, 0.9998,
                                        op0=OP.is_gt, op1=OP.mult)
                i32 = tmpp.tile([128, nj, 128], I32, tag="i32", name="i32")
                nc.vector.scalar_tensor_tensor(i32[:], kf[:], 0.4999, t[:],
                                               op0=OP.add, op1=OP.subtract)
                k16 = b16p.tile([128, nj, 128], qk_dt, tag="k16", name="k16")
                nc.vector.tensor_copy(k16[:], i32[:])
                pst = psT.tile([128, 4, 128], qk_dt, tag="pst", name="pst")
                for j in range(nj):
                    nc.tensor.transpose(pst[:, j, :], k16[:, j, :], ident16[:])
                if dest is None:
                    dest = hd["KT"][c]
                nc.vector.tensor_copy(dest[:], pst[:, 0:nj, :])

            def emit_q(h, hd, c):
                """Cast+transpose Q for chunk c (512 tokens) of head h."""
                s0 = c * 512
                qf = ldp.tile([128, 4, 128], F32, tag="ld")
                nc.sync.dma_start(
                    out=qf[:],
                    in_=q_d[s0:s0 + 512, h, :].rearrange(
                        "(j p) d -> p j d", p=128))
                q16 = b16p.tile([128, 4, 128], qk_dt, tag="q16")
                ceng.tensor_copy(q16[:], qf[:])
                pst2 = psT.tile([128, 4, 128], qk_dt, tag="pst")
                for j in range(4):
                    nc.tensor.transpose(pst2[:, j, :], q16[:, j, :],
                                        ident16[:])
                nc.vector.tensor_copy(hd["QT"][c][:], pst2[:])

            def emit_kq(h, hd, c):
                emit_k(h, hd, c)
                emit_q(h, hd, c)

            def emit_v(h, hd, c, eng=None):
                """Quantize-dequantize V chunk c of head h into Vext."""
                e = eng if eng is not None else nc.vector
                s0 = c * 512
                vf = ldp.tile([128, 4, 128], F32, tag="ld", name="vf")
                nc.sync.dma_start(
                    out=vf[:],
                    in_=v_d[s0:s0 + 512, h, :].rearrange(
                        "(j p) d -> p j d", p=128))
                am2 = smallp.tile([128, 4], F32, tag="am", name="am2")
                # free-axis reduce is DVE-only; cheap (1 of 6 passes)
                nc.vector.reduce_max(am2[:], vf[:], axis=AX,
                                     apply_absolute_value=True)
                sc2 = smallp.tile([128, 4], F32, tag="sc", name="sc2")
                e.tensor_scalar(sc2[:], am2[:], 1e-8, 1.0 / 127.0,
                                op0=OP.max, op1=OP.mult)
                rc2 = smallp.tile([128, 4], F32, tag="rc", name="rc2")
                nc.vector.reciprocal(rc2[:], sc2[:])  # DVE-only op (tiny)
                e.tensor_tensor(vf[:], vf[:], _bcast3(rc2[:], 128),
                                op=OP.mult)
                t2 = tmpp.tile([128, 4, 128], F32, tag="t", name="t2")
                e.tensor_scalar(t2[:], vf[:], 0.0, 0.9998,
                                op0=OP.is_gt, op1=OP.mult)
                i32v = tmpp.tile([128, 4, 128], I32, tag="i32", name="i32v")
                e.scalar_tensor_tensor(i32v[:], vf[:], 0.4999, t2[:],
                                       op0=OP.add, op1=OP.subtract)
                vext = hd["V"]
                e.tensor_tensor(
                    vext[:, 4 * c:4 * c + 4, 0:128], i32v[:],
                    _bcast3(sc2[:], 128), op=OP.mult)
                oeng.memset(vext[:, 4 * c:4 * c + 4, 128:129], 1.0)

            def make_hd(h, split_first=False):
                hd = {
                    "KT": [bigp.tile([128, 512], qk_dt, tag=f"KT{c}",
                                     name=f"KT{c}") for c in range(n_ch)],
                    "QT": [bigp.tile([128, 512], qk_dt, tag=f"QT{c}",
                                     name=f"QT{c}") for c in range(n_ch)],
                    "V": bigp.tile([128, n_kt, 129], BF16, tag="V", name="V"),
                    "ks": [bigp.tile([128, 4], F32, tag=f"ks{c}",
                                     name=f"ks{c}") for c in range(n_ch)],
                    "h": h,
                }
                if split_first:
                    # Halved first-chunk tiles so the very first exp is not
                    # gated on the full 512-token K chain (fill latency).
                    hd["KT0h"] = [bigp.tile([128, 256], qk_dt, tag=f"KT0{i}",
                                            name=f"KT0{i}") for i in range(2)]
                    hd["ks0h"] = [bigp.tile([128, 2], F32, tag=f"ks0{i}",
                                            name=f"ks0{i}") for i in range(2)]
                return hd

            def kt_weight(hd, kt):
                """[128(d), 128(k)] stationary slice + kscale [128,1] AP."""
                if kt < 4 and "KT0h" in hd:
                    tl = hd["KT0h"][kt // 2]
                    ks = hd["ks0h"][kt // 2]
                    return (tl[:, (kt % 2) * 128:(kt % 2 + 1) * 128],
                            ks[:, kt % 2:kt % 2 + 1])
                return (hd["KT"][kt // 4][:, (kt % 4) * 128:(kt % 4 + 1) * 128],
                        hd["ks"][kt // 4][:, kt % 4:kt % 4 + 1])

            def emit_pv(prev, j):
                """Attention-weighted V for query tile j (128 q) of a
                completed (head, qbase, npv*128 cols) score block, +
                denom normalize."""
                pts, vext, h, qbase, _ = prev
                ops_ = psO.tile([128, 129], F32, tag="ops")
                for kt in range(n_kt):
                    nc.tensor.matmul(
                        ops_[:], pts[kt][:, j * 128:(j + 1) * 128],
                        vext[:, kt, 0:129],
                        start=(kt == 0), stop=(kt == n_kt - 1))
                rcp = smallp.tile([128, 1], F32, tag="rcp")
                nc.vector.reciprocal(rcp[:], ops_[:, 128:129])
                if j % 4 == 0:
                    prev_ot[0] = outp.tile([128, 4, 128], F32, tag="ot", name="ot")
                ot = prev_ot[0]
                nc.vector.tensor_scalar(ot[:, j % 4, :], ops_[:, 0:128],
                                        rcp[:], None, op0=OP.mult)
                if j % 4 == 3:
                    q0 = qc * 1024 + (j - 3) * 128
                    nc.sync.dma_start(
                        out=o_d[q0:q0 + 512, h, :].rearrange(
                            "(j p) d -> p j d", p=128),
                        in_=ot[:])

            # Emission schedule: hooks[(h, qc, kt)] -> list of pre-work
            # thunks, spreading head h+1's preprocessing across head h's
            # main loop so the static per-engine schedule zippers instead
            # of head-of-line blocking.  K+Q chunks of head h are needed
            # from that head's qc0; V only from its qc1 (first PV).
            hds = {0: make_hd(0, split_first=split_first)}
            hooks = {}
            kq_pos = [(1, 4), (1, 14), (1, 24), (2, 2), (2, 12), (2, 22),
                      (3, 0), (3, 8)]
            v_pos = [(3, 16), (3, 24)] + [(4, kt) for kt in
                                          (2, 7, 12, 17, 22, 27)]
            for h in range(1, H):
                hds[h] = make_hd(h)
                for c in range(n_ch):
                    qc, kt = kq_pos[c]
                    hooks.setdefault((h - 1, qc, kt), []).append(
                        (emit_kq, h, c))
                for c in range(n_ch):
                    qc, kt = v_pos[c]
                    hq, hqc = (h - 1, qc) if qc < n_qc else (h, qc - n_qc)
                    hooks.setdefault((hq, hqc, kt), []).append(
                        (emit_v, h, c))

            # Head-0 prologue: Q runs one chunk ahead of K (the first QK
            # matmul needs QT chunks 0 AND 1, but only KT chunk 0), V last
            # (first needed by PV at qc1).  The first K chunk is emitted as
            # two 256-token halves so exp(kt0) is not gated on a full
            # 512-token chain.  The first v_pool V chunks run on GpSimd:
            # the DVE prologue (K chains + copies + remaining V) would
            # otherwise finish after the first PV needs Vext.
            hd0 = hds[0]
            if split_first:
                emit_k(0, hd0, 0, j0=0, nj=2, dest=hd0["KT0h"][0],
                       ksdest=hd0["ks0h"][0])
                emit_q(0, hd0, 0)
                emit_q(0, hd0, 1)
                emit_k(0, hd0, 0, j0=2, nj=2, dest=hd0["KT0h"][1],
                       ksdest=hd0["ks0h"][1])
            else:
                emit_k(0, hd0, 0)
                emit_q(0, hd0, 0)
                emit_q(0, hd0, 1)
            for c in range(1, n_ch):
                emit_k(0, hd0, c)
                if c + 1 < n_ch:
                    emit_q(0, hd0, c + 1)
            for c in range(n_ch):
                emit_v(0, hd0, c, eng=nc.gpsimd if c < v_pool else None)

            prev = None          # completed (pts, vext, h, qc) awaiting PV
            prev_ot = [None]
            for h in range(H):
                hd = hds[h]
                for qc in range(n_qc):
                    pts = []
                    for kt in range(n_kt):
                        for fn, hh, cc in hooks.get((h, qc, kt), ()):
                            fn(hh, hds[hh], cc)
                        if (prev is not None and kt % pv_chunk == 0
                                and kt // pv_chunk < 8):
                            emit_pv(prev, kt // pv_chunk)
                        sps = psS.tile([128, 1024], F32, tag="sps")
                        w, ksl = kt_weight(hd, kt)
                        for half in range(2):
                            nc.tensor.matmul(
                                sps[:, half * 512:(half + 1) * 512], w,
                                hd["QT"][2 * qc + half][:],
                                start=True, stop=True)
                        pt = ppool.tile([128, 1024], BF16, tag="pp")
                        nc.scalar.activation(pt[:], sps[:], EXP,
                                             bias=bias_t[:], scale=ksl)
                        pts.append(pt)
                    prev = (pts, hd["V"], hd["h"], qc)
            for j in range(8):
                emit_pv(prev, j)

    nc.compile()
    return nc


def get_nc(**kwargs):
    key = tuple(sorted(kwargs.items()))
    if key not in _NC_CACHE:
        _NC_CACHE[key] = _build_nc(**kwargs)
    return _NC_CACHE[key]


def kernel(q, k, v, _trace=False, _trace_cores=None, _nc_kwargs=None):
    """Full-input entry point: q/k/v [1, 4096, 16, 128] fp32 -> same shape."""
    assert q.shape == (1, _S, _H_TOTAL, _D), q.shape
    nc = get_nc(**(_nc_kwargs or {}))
    in_maps = []
    for c in range(_N_CORES):
        hs = slice(c * _H, (c + 1) * _H)
        in_maps.append({
            "q": np.ascontiguousarray(q[0, :, hs, :], dtype=np.float32),
            "k": np.ascontiguousarray(k[0, :, hs, :], dtype=np.float32),
            "v": np.ascontiguousarray(v[0, :, hs, :], dtype=np.float32),
        })
    # The axon-tunneled device occasionally reports a transient
    # NRT_EXEC_UNIT_UNRECOVERABLE on the first execution; a retry succeeds.
    last_err = None
    for attempt in range(3):
        try:
            res = run_bass_kernel_spmd(nc, in_maps,
                                       core_ids=list(range(_N_CORES)),
                                       trace=_trace, trace_cores=_trace_cores)
            break
        except Exception as e:  # noqa: BLE001
            last_err = e
            time.sleep(2.0 * (attempt + 1))
    else:
        raise last_err
    out = np.concatenate([res.results[c]["o"] for c in range(_N_CORES)],
                         axis=1)[None]
    out = np.ascontiguousarray(out, dtype=np.float32)
    if _trace:
        return out, res
    return out


# revision 30
# speedup vs baseline: 1.0043x; 1.0030x over previous
"""ChunkedAttention (nn_ChunkedAttention_43568148251092) Trainium2 kernel.

Full inputs q/k/v: [1, 4096, 16, 128] fp32. Shards the 16 heads across the
8 NeuronCores (2 heads per core, pure head parallelism — no collectives),
runs a Bass/Tile attention kernel per core, and concatenates the results.

Per-head pipeline on each core (S=4096 tokens, D=128):
  - int8 quant-dequant of K and V per token, trunc-toward-zero exactly as the
    reference. Trunc via sign-offset + RNE int convert: t = (x>0)*0.9998;
    i = rne_i32(x + 0.4999 - t). Exact for integer x (incl +/-127); off-by-one
    only for |frac| within 1e-4 of 1, which is numerically negligible.
    Kint kept as fp16 integers (exact: |int| <= 127); per-token kscale kept
    fp32 and folded into the softmax exp via the ACT per-partition scale.
    V dequantized to bf16 (Vint * vscale).
  - Q cast to fp16; Q and Kint transposed to [d, s] via PE transpose.
  - S^T[k,q] = Kint^T.T @ Q^T in PSUM fp32 (single 128-deep matmul).
  - P'[k,q] = exp(kscale/sqrt(D) * S^T - 40) via ScalarE (bias keeps the
    bf16 range safe without a row-max pass; scaled scores are ~N(0,1)).
  - out[q, 0:128|denom] = sum_kt P'_kt.T @ [Vdq | ones] accumulated in PSUM;
    the appended ones-column yields the softmax denominator for free.
  - out = out[:, :128] * (1/denom) per partition, DMA to DRAM.

Scheduling structure (the point of this rewrite vs the first version):
  - per-head big tiles are double-buffered so head h+1's preprocessing
    (DVE-heavy) overlaps head h's main loop (ACT-heavy exp is the
    bottleneck engine: 256 x [128,1024] exps ~= 266us busy).
  - PSUM->SBUF transpose copies go explicitly to DVE (nc.any would put
    them on ScalarE, polluting the bottleneck engine).
  - Q fp32->fp16 casts and Vext ones-memsets go to GpSimd (idle; SBUF-only).
  - PV matmuls for qc are interleaved into the QK/exp stream of qc+1 so
    PE stays dense while ACT runs exps.
  - output stores batched 512 tokens per DMA.
"""

import math
import time

import numpy as np

import concourse.bass as bass
import concourse.mybir as mybir
import concourse.tile as tile
from concourse import bacc
from concourse.bass_utils import run_bass_kernel_spmd
from concourse.masks import make_identity

F32 = mybir.dt.float32
BF16 = mybir.dt.bfloat16
FP16 = mybir.dt.float16
I32 = mybir.dt.int32
AX = mybir.AxisListType.X
OP = mybir.AluOpType
EXP = mybir.ActivationFunctionType.Exp

_S = 4096
_H_TOTAL = 16
_D = 128
_N_CORES = 8
_H = _H_TOTAL // _N_CORES  # heads per core

_NC_CACHE = {}


def _bcast3(ap2, n):
    """[128, J] AP -> [128, J, n] broadcast AP (inner stride 0)."""
    return bass.AP(tensor=ap2.tensor, offset=ap2.offset, ap=[*ap2.ap, [0, n]])


def _build_nc(S=_S, H=_H, D=_D, qk_dt=FP16, pp_bufs=63, ld_bufs=6,
              psS_bufs=2, psT_bufs=1, psO_bufs=3, pre_emit="qc0",
              pv_chunk=4, cast_eng="gpsimd", ones_eng="gpsimd",
              split_first=True, v_pool=0):
    assert D == 128 and S % 512 == 0
    n_kt = S // 128          # 32 key tiles of 128 tokens
    n_ch = S // 512          # 8 chunks of 512 tokens
    n_qc = S // 1024         # 4 query column blocks of 1024

    nc = bacc.Bacc("TRN2")
    q_d = nc.dram_tensor("q", [S, H, D], F32, kind="ExternalInput")
    k_d = nc.dram_tensor("k", [S, H, D], F32, kind="ExternalInput")
    v_d = nc.dram_tensor("v", [S, H, D], F32, kind="ExternalInput")
    o_d = nc.dram_tensor("o", [S, H, D], F32, kind="ExternalOutput")

    with tile.TileContext(nc) as tc:
        with (
            tc.tile_pool(name="const", bufs=1) as constp,
            tc.tile_pool(name="big", bufs=2) as bigp,
            tc.tile_pool(name="ld", bufs=ld_bufs) as ldp,
            tc.tile_pool(name="tmp", bufs=2) as tmpp,
            tc.tile_pool(name="b16", bufs=2) as b16p,
            tc.tile_pool(name="small", bufs=8) as smallp,
            tc.tile_pool(name="pp", bufs=pp_bufs) as ppool,
            tc.tile_pool(name="outp", bufs=2) as outp,
            tc.tile_pool(name="psT", bufs=psT_bufs, space="PSUM") as psT,
            tc.tile_pool(name="psS", bufs=psS_bufs, space="PSUM") as psS,
            tc.tile_pool(name="psO", bufs=psO_bufs, space="PSUM") as psO,
        ):
            bias_t = constp.tile([128, 1], F32)
            nc.vector.memset(bias_t[:], -40.0)
            # Dummy activation emitted first so the ACT table load happens
            # at t~0 instead of just before the first real exp.
            warm = constp.tile([128, 1], F32)
            nc.scalar.activation(warm[:], bias_t[:], EXP, bias=bias_t[:])
            ident32 = constp.tile([128, 128], F32)
            make_identity(nc, ident32[:])
            ident16 = constp.tile([128, 128], qk_dt)
            nc.vector.tensor_copy(ident16[:], ident32[:])
            ceng = getattr(nc, cast_eng)
            oeng = getattr(nc, ones_eng)

            def emit_k(h, hd, c, j0=0, nj=4, dest=None, ksdest=None):
                """Quantize K tokens [c*512+j0*128, +nj*128) of head h into
                dest (default: the chunk's KT tile) + kscale tile."""
                s0 = c * 512 + j0 * 128
                kf = ldp.tile([128, nj, 128], F32, tag="ld", name="kf")
                nc.sync.dma_start(
                    out=kf[:],
                    in_=k_d[s0:s0 + nj * 128, h, :].rearrange(
                        "(j p) d -> p j d", p=128))
                am = smallp.tile([128, nj], F32, tag="am", name="am")
                nc.vector.reduce_max(am[:], kf[:], axis=AX,
                                     apply_absolute_value=True)
                sc = smallp.tile([128, nj], F32, tag="sc", name="sc")
                nc.vector.tensor_scalar(sc[:], am[:], 1e-8, 1.0 / 127.0,
                                        op0=OP.max, op1=OP.mult)
                ks = ksdest if ksdest is not None else hd["ks"][c]
                nc.vector.tensor_scalar(ks[:], sc[:], 1.0 / math.sqrt(128.0),
                                        None, op0=OP.mult)
                rc = smallp.tile([128, nj], F32, tag="rc", name="rc")
                nc.vector.reciprocal(rc[:], sc[:])
                nc.vector.tensor_tensor(kf[:], kf[:], _bcast3(rc[:], 128),
                                        op=OP.mult)  # x, in-place
                t = tmpp.tile([128, nj, 128], F32, tag="t", name="t")
                nc.vector.tensor_scalar(t[:], kf[:], 0.0, 0.9998,
                                        op0=OP.is_gt, op1=OP.mult)
                i32 = tmpp.tile([128, nj, 128], I32, tag="i32", name="i32")
                nc.vector.scalar_tensor_tensor(i32[:], kf[:], 0.4999, t[:],
                                               op0=OP.add, op1=OP.subtract)
                k16 = b16p.tile([128, nj, 128], qk_dt, tag="k16", name="k16")
                nc.vector.tensor_copy(k16[:], i32[:])
                pst = psT.tile([128, 4, 128], qk_dt, tag="pst", name="pst")
                for j in range(nj):
                    nc.tensor.transpose(pst[:, j, :], k16[:, j, :], ident16[:])
                if dest is None:
                    dest = hd["KT"][c]
                nc.vector.tensor_copy(dest[:], pst[:, 0:nj, :])

            def emit_q(h, hd, c):
                """Cast+transpose Q for chunk c (512 tokens) of head h."""
                s0 = c * 512
                qf = ldp.tile([128, 4, 128], F32, tag="ld")
                nc.sync.dma_start(
                    out=qf[:],
                    in_=q_d[s0:s0 + 512, h, :].rearrange(
                        "(j p) d -> p j d", p=128))
                q16 = b16p.tile([128, 4, 128], qk_dt, tag="q16")
                ceng.tensor_copy(q16[:], qf[:])
                pst2 = psT.tile([128, 4, 128], qk_dt, tag="pst")
                for j in range(4):
                    nc.tensor.transpose(pst2[:, j, :], q16[:, j, :],
                                        ident16[:])
                nc.vector.tensor_copy(hd["QT"][c][:], pst2[:])

            def emit_kq(h, hd, c):
                emit_k(h, hd, c)
                emit_q(h, hd, c)

            def emit_v(h, hd, c, eng=None):
                """Quantize-dequantize V chunk c of head h into Vext."""
                e = eng if eng is not None else nc.vector
                s0 = c * 512
                vf = ldp.tile([128, 4, 128], F32, tag="ld", name="vf")
                nc.sync.dma_start(
                    out=vf[:],
                    in_=v_d[s0:s0 + 512, h, :].rearrange(
                        "(j p) d -> p j d", p=128))
                am2 = smallp.tile([128, 4], F32, tag="am", name="am2")
                # free-axis reduce is DVE-only; cheap (1 of 6 passes)
                nc.vector.reduce_max(am2[:], vf[:], axis=AX,
                                     apply_absolute_value=True)
                sc2 = smallp.tile([128, 4], F32, tag="sc", name="sc2")
                e.tensor_scalar(sc2[:], am2[:], 1e-8, 1.0 / 127.0,
                                op0=OP.max, op1=OP.mult)
                rc2 = smallp.tile([128, 4], F32, tag="rc", name="rc2")
                nc.vector.reciprocal(rc2[:], sc2[:])  # DVE-only op (tiny)
                e.tensor_tensor(vf[:], vf[:], _bcast3(rc2[:], 128),
                                op=OP.mult)
                t2 = tmpp.tile([128, 4, 128], F32, tag="t", name="t2")
                e.tensor_scalar(t2[:], vf[:], 0.0, 0.9998,
                                op0=OP.is_gt, op1=OP.mult)
                i32v = tmpp.tile([128, 4, 128], I32, tag="i32", name="i32v")
                e.scalar_tensor_tensor(i32v[:], vf[:], 0.4999, t2[:],
                                       op0=OP.add, op1=OP.subtract)
                vext = hd["V"]
                e.tensor_tensor(
                    vext[:, 4 * c:4 * c + 4, 0:128], i32v[:],
                    _bcast3(sc2[:], 128), op=OP.mult)
                oeng.memset(vext[:, 4 * c:4 * c + 4, 128:129], 1.0)

            def make_hd(h, split_first=False):
                hd = {
                    "KT": [bigp.tile([128, 512], qk_dt, tag=f"KT{c}",
                                     name=f"KT{c}") for c in range(n_ch)],
                    "QT": [bigp.tile([128, 512], qk_dt, tag=f"QT{c}",
                                     name=f"QT{c}") for c in range(n_ch)],
                    "V": bigp.tile([128, n_kt, 129], BF16, tag="V", name="V"),
                    "ks": [bigp.tile([128, 4], F32, tag=f"ks{c}",
                                     name=f"ks{c}") for c in range(n_ch)],
                    "h": h,
                }
                if split_first:
                    # Halved first-chunk tiles so the very first exp is not
                    # gated on the full 512-token K chain (fill latency).
                    hd["KT0h"] = [bigp.tile([128, 256], qk_dt, tag=f"KT0{i}",
                                            name=f"KT0{i}") for i in range(2)]
                    hd["ks0h"] = [bigp.tile([128, 2], F32, tag=f"ks0{i}",
                                            name=f"ks0{i}") for i in range(2)]
                return hd

            def kt_weight(hd, kt):
                """[128(d), 128(k)] stationary slice + kscale [128,1] AP."""
                if kt < 4 and "KT0h" in hd:
                    tl = hd["KT0h"][kt // 2]
                    ks = hd["ks0h"][kt // 2]
                    return (tl[:, (kt % 2) * 128:(kt % 2 + 1) * 128],
                            ks[:, kt % 2:kt % 2 + 1])
                return (hd["KT"][kt // 4][:, (kt % 4) * 128:(kt % 4 + 1) * 128],
                        hd["ks"][kt // 4][:, kt % 4:kt % 4 + 1])

            def emit_pv(prev, j):
                """Attention-weighted V for query tile j (128 q) of a
                completed (head, qbase, npv*128 cols) score block, +
                denom normalize."""
                pts, vext, h, qbase, _ = prev
                ops_ = psO.tile([128, 129], F32, tag="ops")
                for kt in range(n_kt):
                    nc.tensor.matmul(
                        ops_[:], pts[kt][:, j * 128:(j + 1) * 128],
                        vext[:, kt, 0:129],
                        start=(kt == 0), stop=(kt == n_kt - 1))
                rcp = smallp.tile([128, 1], F32, tag="rcp")
                nc.vector.reciprocal(rcp[:], ops_[:, 128:129])
                if j % 4 == 0:
                    prev_ot[0] = outp.tile([128, 4, 128], F32, tag="ot", name="ot")
                ot = prev_ot[0]
                nc.vector.tensor_scalar(ot[:, j % 4, :], ops_[:, 0:128],
                                        rcp[:], None, op0=OP.mult)
                if j % 4 == 3:
                    q0 = qbase + (j - 3) * 128
                    nc.sync.dma_start(
                        out=o_d[q0:q0 + 512, h, :].rearrange(
                            "(j p) d -> p j d", p=128),
                        in_=ot[:])

            # Emission schedule: hooks[(h, qc, kt)] -> list of pre-work
            # thunks, spreading head h+1's preprocessing across head h's
            # main loop so the static per-engine schedule zippers instead
            # of head-of-line blocking.  K+Q chunks of head h are needed
            # from that head's qc0; V only from its qc1 (first PV).
            hds = {0: make_hd(0, split_first=split_first)}
            hooks = {}
            kq_pos = [(1, 4), (1, 14), (1, 24), (2, 2), (2, 12), (2, 22),
                      (3, 0), (3, 8)]
            v_pos = [(3, 16), (3, 24)] + [(4, kt) for kt in
                                          (2, 7, 12, 17, 22, 27)]
            for h in range(1, H):
                hds[h] = make_hd(h)
                for c in range(n_ch):
                    qc, kt = kq_pos[c]
                    hooks.setdefault((h - 1, qc, kt), []).append(
                        (emit_kq, h, c))
                for c in range(n_ch):
                    qc, kt = v_pos[c]
                    hq, hqc = (h - 1, qc) if qc < n_qc else (h, qc - n_qc)
                    hooks.setdefault((hq, hqc, kt), []).append(
                        (emit_v, h, c))

            # Head-0 prologue: Q runs one chunk ahead of K (the first QK
            # matmul needs QT chunks 0 AND 1, but only KT chunk 0), V last
            # (first needed by PV at qc1).  The first K chunk is emitted as
            # two 256-token halves so exp(kt0) is not gated on a full
            # 512-token chain.  The first v_pool V chunks run on GpSimd:
            # the DVE prologue (K chains + copies + remaining V) would
            # otherwise finish after the first PV needs Vext.
            hd0 = hds[0]
            if split_first:
                emit_k(0, hd0, 0, j0=0, nj=2, dest=hd0["KT0h"][0],
                       ksdest=hd0["ks0h"][0])
                emit_q(0, hd0, 0)
                emit_q(0, hd0, 1)
                emit_k(0, hd0, 0, j0=2, nj=2, dest=hd0["KT0h"][1],
                       ksdest=hd0["ks0h"][1])
            else:
                emit_k(0, hd0, 0)
                emit_q(0, hd0, 0)
                emit_q(0, hd0, 1)
            for c in range(1, n_ch):
                emit_k(0, hd0, c)
                if c + 1 < n_ch:
                    emit_q(0, hd0, c + 1)
            for c in range(n_ch):
                emit_v(0, hd0, c, eng=nc.gpsimd if c < v_pool else None)

            prev = None       # completed (pts, vext, h, qbase, npv) block
            prev_ot = [None]

            def maybe_pv(kt):
                if prev is None:
                    return
                npv = prev[4]
                sp = n_kt // npv
                if kt % sp == 0 and kt // sp < npv:
                    emit_pv(prev, kt // sp)

            for h in range(H):
                hd = hds[h]
                for qc in range(n_qc):
                    last = (h == H - 1 and qc == n_qc - 1)
                    if not last:
                        pts = []
                        for kt in range(n_kt):
                            for fn, hh, cc in hooks.get((h, qc, kt), ()):
                                fn(hh, hds[hh], cc)
                            maybe_pv(kt)
                            sps = psS.tile([128, 1024], F32, tag="sps")
                            w, ksl = kt_weight(hd, kt)
                            for half in range(2):
                                nc.tensor.matmul(
                                    sps[:, half * 512:(half + 1) * 512], w,
                                    hd["QT"][2 * qc + half][:],
                                    start=True, stop=True)
                            pt = ppool.tile([128, 1024], BF16, tag="pp")
                            nc.scalar.activation(pt[:], sps[:], EXP,
                                                 bias=bias_t[:], scale=ksl)
                            pts.append(pt)
                        prev = (pts, hd["V"], hd["h"], qc * 1024, 8)
                    else:
                        # Final block: two 512-col halves so the closing PV
                        # chain (otherwise ~14us of pure tail) halves, with
                        # half A's PV overlapping half B's exps.
                        for half in range(2):
                            pts = []
                            for kt in range(n_kt):
                                maybe_pv(kt)
                                sps = psS.tile([128, 1024], F32, tag="sps")
                                w, ksl = kt_weight(hd, kt)
                                nc.tensor.matmul(
                                    sps[:, 0:512], w,
                                    hd["QT"][2 * qc + half][:],
                                    start=True, stop=True)
                                pt = ppool.tile([128, 512], BF16, tag="pp",
                                                name="pth")
                                nc.scalar.activation(pt[:], sps[:, 0:512],
                                                     EXP, bias=bias_t[:],
                                                     scale=ksl)
                                pts.append(pt)
                            prev = (pts, hd["V"], hd["h"],
                                    qc * 1024 + half * 512, 4)
            for j in range(prev[4]):
                emit_pv(prev, j)

    nc.compile()
    return nc


def get_nc(**kwargs):
    key = tuple(sorted(kwargs.items()))
    if key not in _NC_CACHE:
        _NC_CACHE[key] = _build_nc(**kwargs)
    return _NC_CACHE[key]


def kernel(q, k, v, _trace=False, _trace_cores=None, _nc_kwargs=None):
    """Full-input entry point: q/k/v [1, 4096, 16, 128] fp32 -> same shape."""
    assert q.shape == (1, _S, _H_TOTAL, _D), q.shape
    nc = get_nc(**(_nc_kwargs or {}))
    in_maps = []
    for c in range(_N_CORES):
        hs = slice(c * _H, (c + 1) * _H)
        in_maps.append({
            "q": np.ascontiguousarray(q[0, :, hs, :], dtype=np.float32),
            "k": np.ascontiguousarray(k[0, :, hs, :], dtype=np.float32),
            "v": np.ascontiguousarray(v[0, :, hs, :], dtype=np.float32),
        })
    # The axon-tunneled device occasionally reports a transient
    # NRT_EXEC_UNIT_UNRECOVERABLE on the first execution; a retry succeeds.
    last_err = None
    for attempt in range(3):
        try:
            res = run_bass_kernel_spmd(nc, in_maps,
                                       core_ids=list(range(_N_CORES)),
                                       trace=_trace, trace_cores=_trace_cores)
            break
        except Exception as e:  # noqa: BLE001
            last_err = e
            time.sleep(2.0 * (attempt + 1))
    else:
        raise last_err
    out = np.concatenate([res.results[c]["o"] for c in range(_N_CORES)],
                         axis=1)[None]
    out = np.ascontiguousarray(out, dtype=np.float32)
    if _trace:
        return out, res
    return out
